# revision 18
# baseline (speedup 1.0000x reference)
"""BigBird ViT forward on 8 Trainium2 NeuronCores — half-staggered attention.

Sharding: every core holds 136 tokens of BOTH batch elements (8-way token
sharding per element). Dense compute (LN / projections / FFN) runs fused over
all 272 resident token columns; K/V projections + the per-element fused K/V
AllGather run per batch element and are staggered so each AllGather hides
behind the other element's projections / Q / attention.

On-chip layout is transposed ([feature, token]); each 272-wide token axis is
[136 tokens of b0 | 136 tokens of b1]. LayerNorm reductions run as a single
f32r ones-matmul; rstd = exp(-0.5*ln(var+eps)). LN gain/bias are folded into
the following projections host-side. BigBird band/random/global structure is
a multiplicative {0,1,2} bf16 mask. Attention is software-pipelined: head
h+1's QK matmuls are emitted before head h's PV so the PE never waits on the
exp/mask chain.
"""
import os
import sys

sys.path.insert(0, "/opt/trn_rl_repo")

import numpy as np
import ml_dtypes

import concourse.bass as bass
import concourse.bacc as bacc
import concourse.mybir as mybir
import concourse.tile as tile
from concourse.bass_utils import run_bass_kernel_spmd

F32 = mybir.dt.float32
F32R = mybir.dt.float32r
BF16 = mybir.dt.bfloat16
F8 = mybir.dt.float8e4
AF = mybir.ActivationFunctionType
ALU = mybir.AluOpType
BF = ml_dtypes.bfloat16

# model dims
BS = 64; NH = 12; HD = 64; D = 768; F = 3072; L = 12; R = 3
SEQ = 1025
SEQP = 1088           # padded to 17 blocks of 64
NBLK = 17
TH = SEQP // 8        # tokens per core per batch element = 136
T = 2 * TH            # token columns per core = 272 (b0 | b1)
DT = D // 128         # 6 feature tiles
FT = F // 128         # 24 ffn tiles
KT = 9                # k tiles over 1152 (1088 padded up; tile 8 is half real)
KPAD = 1152
VCOLS = NH * (HD + 1)  # 780: per-head [64 V cols + 1 ones col]
SC = 1.0 / np.sqrt(HD)
LNP = 48              # bias columns per layer
G8 = [[0, 1, 2, 3, 4, 5, 6, 7]]
KB = DT * TH          # 816: K bounce cols per half

NLAYERS = int(os.environ.get("BB_NLAYERS", str(L)))

_CACHE = {}


# ---------------------------------------------------------------- builder

def build_program(nlayers=NLAYERS):
    nc = bacc.Bacc("TRN2", target_bir_lowering=False, debug=False, num_devices=8)

    # ---- DRAM I/O -------------------------------------------------------
    pe_in = nc.dram_tensor("pe_in", [128, DT * T], BF16, kind="ExternalInput")
    add_in = nc.dram_tensor("add_in", [128, DT * T], F32, kind="ExternalInput")
    mask_in = nc.dram_tensor("mask_in", [128, NH * (KT - 1) * TH], BF16, kind="ExternalInput")
    pw_in = nc.dram_tensor("pw", [D, D], BF16, kind="ExternalInput")
    normp_in = nc.dram_tensor("normp", [128, 2 * DT], F32, kind="ExternalInput")
    lnp_in = nc.dram_tensor("lnp", [128, LNP * nlayers], F32, kind="ExternalInput")
    wq = [nc.dram_tensor(f"wq{i}", [D, D], BF16, kind="ExternalInput") for i in range(nlayers)]
    wk = [nc.dram_tensor(f"wk{i}", [D, D], BF16, kind="ExternalInput") for i in range(nlayers)]
    wv = [nc.dram_tensor(f"wv{i}", [D, VCOLS], BF16, kind="ExternalInput") for i in range(nlayers)]
    wo = [nc.dram_tensor(f"wo{i}", [D, D], BF16, kind="ExternalInput") for i in range(nlayers)]
    w1 = [nc.dram_tensor(f"w1{i}", [128, FT * D], BF16, kind="ExternalInput") for i in range(nlayers)]
    w2 = [nc.dram_tensor(f"w2{i}", [128, DT * F], BF16, kind="ExternalInput") for i in range(nlayers)]
    bvb = [nc.dram_tensor(f"bvb{i}", [128, VCOLS], BF16, kind="ExternalInput") for i in range(nlayers)]
    out_t = nc.dram_tensor("out", [128, DT * T], F32, kind="ExternalOutput")

    KV_K = D * TH
    KV_V = TH * VCOLS
    KV = KV_K + KV_V
    kvi = [nc.dram_tensor(f"kv_in{b}", [KV], F8) for b in range(2)]
    kvo = [nc.dram_tensor(f"kv_out{b}", [8 * KV], F8, addr_space="Shared")
           for b in range(2)]

    with tile.TileContext(nc) as tc:
        # ---- persistent SBUF tensors -----------------------------------
        X = nc.alloc_sbuf_tensor("X", [128, DT * T], F32R)
        xn = nc.alloc_sbuf_tensor("xn", [128, DT * T], BF16)
        Qt = nc.alloc_sbuf_tensor("Qt", [128, DT * T], BF16)
        Ktl = nc.alloc_sbuf_tensor("Ktl", [128, 2 * KB], F8)   # [half][t][j]
        Vnl = nc.alloc_sbuf_tensor("Vnl", [128, 4 * VCOLS], F8)  # [half][tile 128|8]
        Ktf = [nc.alloc_sbuf_tensor(f"Ktf{b}", [128, DT * KPAD], F8) for b in range(2)]
        Vnf = [nc.alloc_sbuf_tensor(f"Vnf{b}", [128, KT * VCOLS], F8) for b in range(2)]
        ctx = nc.alloc_sbuf_tensor("ctx", [128, DT * T], BF16)
        hsb = nc.alloc_sbuf_tensor("hsb", [128, FT * T], BF16)
        msk = nc.alloc_sbuf_tensor("msk", [128, NH * (KT - 1) * TH], BF16)
        m8c = nc.alloc_sbuf_tensor("m8c", [128, TH], BF16)
        onesr = nc.alloc_sbuf_tensor("onesr", [128, 1], F32R)
        onesf = nc.alloc_sbuf_tensor("onesf", [128, 1], F32)
        lnp_sb = nc.alloc_sbuf_tensor("lnp_sb", [128, LNP * nlayers], F32)
        normp_sb = nc.alloc_sbuf_tensor("normp_sb", [128, 2 * DT], F32)
        epsb = nc.alloc_sbuf_tensor("epsb", [1, 1], F32)

        with (
            tc.tile_pool(name="wpool", bufs=3) as wpool,
            tc.tile_pool(name="w1pool", bufs=2) as w1pool,
            tc.tile_pool(name="w2pool", bufs=2) as w2pool,
            tc.tile_pool(name="bvpool", bufs=2) as bvpool,
            tc.tile_pool(name="work", bufs=2) as work,
            tc.tile_pool(name="stat", bufs=2) as stat,
            tc.tile_pool(name="ppool", bufs=4) as ppool,
            tc.tile_pool(name="psq", bufs=6, space="PSUM") as psq,
            tc.tile_pool(name="psa", bufs=2, space="PSUM") as psa,
        ):
            nc.vector.memset(onesf[:], 1.0)
            nc.vector.tensor_copy(onesr[:], onesf[:])
            nc.vector.memset(epsb[:], 1e-5)
            for b in range(2):
                for t in range(DT):
                    nc.vector.memset(Ktf[b][:, t * KPAD + SEQP: (t + 1) * KPAD], 0.0)
                nc.vector.memset(Vnf[b][64:128, (KT - 1) * VCOLS: KT * VCOLS], 0.0)
            nc.sync.dma_start(out=msk[:], in_=mask_in[:, :])
            nc.sync.dma_start(out=lnp_sb[:], in_=lnp_in[:, :])
            nc.sync.dma_start(out=normp_sb[:], in_=normp_in[:, :])
            nc.vector.memset(m8c[:], 0.0)
            nc.vector.memset(m8c[0:1, :], 1.0)

            def layernorm(out_sb, base, w, final=False):
                """fused LN over features for token cols [base, base+w).

                xn = X*rstd - mu*rstd; the elementwise work is split between
                the Vector and GpSimd engines; mu*rstd | rstd are broadcast in
                one shot to shorten the stat-chain latency.
                """
                sum_ps = psa.tile([1, 512], F32, tag="acc")
                sq_ps = psa.tile([1, 512], F32, tag="acc")
                for t in range(DT):
                    nc.tensor.matmul(sum_ps[:, 0:w], onesr[:],
                                     X[:, t * T + base: t * T + base + w],
                                     start=(t == 0), stop=(t == DT - 1))
                for t in range(DT):
                    sq = work.tile([128, T], F32R, tag="lnsq")
                    s32 = X[:, t * T + base: t * T + base + w].bitcast(F32)
                    eng = nc.vector if t % 2 == 0 else nc.gpsimd
                    eng.tensor_mul(sq[:, 0:w], s32, s32)
                    nc.tensor.matmul(sq_ps[:, 0:w], onesr[:], sq[:, 0:w],
                                     start=(t == 0), stop=(t == DT - 1))
                st = stat.tile([1, 2 * T], F32, tag="mr")   # [mu*rstd | rstd]
                va = stat.tile([1, T], F32, tag="var")
                nc.scalar.activation(st[:, 0:w], sum_ps[:, 0:w], AF.Identity, scale=1.0 / D)
                nc.scalar.activation(va[:, 0:w], sq_ps[:, 0:w], AF.Identity, scale=1.0 / D)
                mu2 = stat.tile([1, T], F32, tag="mu2")
                nc.vector.tensor_mul(mu2[:, 0:w], st[:, 0:w], st[:, 0:w])
                nc.vector.tensor_sub(va[:, 0:w], va[:, 0:w], mu2[:, 0:w])
                nc.scalar.activation(va[:, 0:w], va[:, 0:w], AF.Ln, bias=epsb[0:1, 0:1])
                nc.scalar.activation(st[:, w:2 * w], va[:, 0:w], AF.Exp, scale=-0.5)
                nc.vector.tensor_mul(st[:, 0:w], st[:, 0:w], st[:, w:2 * w])
                mr2 = work.tile([128, 2 * T], F32, tag="mrb")
                nc.gpsimd.partition_broadcast(mr2[:, 0:2 * w], st[:, 0:2 * w])
                for t in range(DT):
                    eng = nc.vector if t % 2 == 0 else nc.gpsimd
                    tmp = work.tile([128, T], F32, tag="lntmp")
                    eng.tensor_mul(tmp[:, 0:w],
                                   X[:, t * T + base: t * T + base + w].bitcast(F32),
                                   mr2[:, w:2 * w])
                    if final:
                        eng.tensor_sub(tmp[:, 0:w], tmp[:, 0:w], mr2[:, 0:w])
                        nc.vector.tensor_scalar(
                            out_sb[:, t * T + base: t * T + base + w], tmp[:, 0:w],
                            normp_sb[:, t:t + 1], normp_sb[:, DT + t:DT + t + 1],
                            op0=ALU.mult, op1=ALU.add)
                    else:
                        eng.tensor_sub(out_sb[:, t * T + base: t * T + base + w],
                                       tmp[:, 0:w], mr2[:, 0:w])

            def load_slab(wdram, ncols):
                slab = wpool.tile([128, DT * VCOLS], BF16, tag="slab")
                nc.sync.dma_start(
                    out=slab[:, 0:DT * ncols].rearrange("p (t c) -> p t c", c=ncols),
                    in_=wdram[:, :].rearrange("(t p) c -> p t c", p=128))
                return slab

            def proj_half(slab, out_sb, bias_col, base, w, half_major=False):
                """out[:, o-tile cols base..base+w] = slab.T @ xn cols."""
                for o in range(DT):
                    psm = psa.tile([128, 512], F32, tag="acc")
                    for t in range(DT):
                        nc.tensor.matmul(psm[:, 0:w],
                                         slab[:, t * D + o * 128: t * D + (o + 1) * 128],
                                         xn[:, t * T + base: t * T + base + w],
                                         start=(t == 0), stop=(t == DT - 1))
                    if half_major:
                        dst = out_sb[:, base * DT + o * w: base * DT + (o + 1) * w]
                    else:
                        dst = out_sb[:, o * T + base: o * T + base + w]
                    nc.vector.tensor_scalar(
                        dst, psm[:, 0:w],
                        lnp_sb[:, bias_col + o:bias_col + o + 1], None,
                        op0=ALU.add)

            def vproj_half(vslab, bvs, b):
                """V proj for half b into Vnl tiles (2b: rows0-127, 2b+1: rows 0-7)."""
                base = b * TH
                for s, rows in ((0, 128), (1, TH - 128)):
                    for half in range(2):
                        n0, n1 = half * 390, (half + 1) * 390
                        psm = psa.tile([128, 512], F32, tag="acc")
                        for t in range(DT):
                            nc.tensor.matmul(
                                psm[:rows, 0:390],
                                xn[:, t * T + base + s * 128: t * T + base + s * 128 + rows],
                                vslab[:, t * VCOLS + n0: t * VCOLS + n1],
                                start=(t == 0), stop=(t == DT - 1))
                        nc.vector.tensor_add(
                            Vnl[:rows, (2 * b + s) * VCOLS + n0: (2 * b + s) * VCOLS + n1],
                            psm[:rows, 0:390], bvs[:rows, n0:n1])

            def bounce(b):
                """ship half b's K^T / V shards to the collective input."""
                nc.sync.dma_start(
                    out=kvi[b][0:KV_K].rearrange("(p x) -> p x", p=128),
                    in_=Ktl[:, b * KB:(b + 1) * KB])
                nc.sync.dma_start(
                    out=kvi[b][KV_K:KV_K + 128 * VCOLS].rearrange("(a v) -> a v", v=VCOLS),
                    in_=Vnl[0:128, 2 * b * VCOLS:(2 * b + 1) * VCOLS])
                nc.sync.dma_start(
                    out=kvi[b][KV_K + 128 * VCOLS:KV].rearrange("(a v) -> a v", v=VCOLS),
                    in_=Vnl[0:TH - 128, (2 * b + 1) * VCOLS:(2 * b + 2) * VCOLS])

            def gather(b):
                nc.gpsimd.collective_compute(
                    "AllGather", ALU.bypass, replica_groups=G8,
                    ins=[kvi[b][:].opt()], outs=[kvo[b][:].opt()])

            def assemble(b):
                for c in range(8):
                    kc = kvo[b][c * KV: c * KV + KV_K]
                    nc.sync.dma_start(
                        out=Ktf[b][:].rearrange("p (t k) -> p t k", k=KPAD)
                            [:, :, c * TH:(c + 1) * TH],
                        in_=kc.rearrange("(p t j) -> p t j", t=DT, j=TH))
                for m in range(KT):
                    r0 = m * 128
                    rows = 128 if m < KT - 1 else SEQP - r0
                    r = r0
                    while r < r0 + rows:
                        c = r // TH
                        take = min((c + 1) * TH, r0 + rows) - r
                        vc = kvo[b][c * KV + KV_K + (r - c * TH) * VCOLS:
                                    c * KV + KV_K + (r - c * TH + take) * VCOLS]
                        nc.sync.dma_start(
                            out=Vnf[b][r - r0: r - r0 + take, m * VCOLS:(m + 1) * VCOLS],
                            in_=vc.rearrange("(a v) -> a v", v=VCOLS))
                        r += take

            def attention(b):
                base = b * TH
                pend = None

                def emit_pv(h, P):
                    ft, row = h // 2, (h % 2) * 64
                    cps = psa.tile([65, 512], F32, tag="acc")
                    for m in range(KT):
                        vh = Vnf[b][:, m * VCOLS + h * 65: m * VCOLS + (h + 1) * 65]
                        nc.tensor.matmul(cps[:, 0:TH], vh, P[:, m * TH:(m + 1) * TH],
                                         start=(m == 0), stop=(m == KT - 1))
                    zrec = stat.tile([1, TH], F32, tag="zrec")
                    nc.vector.reciprocal(zrec[:], cps[64:65, 0:TH])
                    zbs = work.tile([64, TH], F32, tag="zbs")
                    nc.gpsimd.partition_broadcast(zbs[:], zrec[:])
                    nc.vector.tensor_mul(ctx[row:row + 64, ft * T + base: ft * T + base + TH],
                                         cps[0:64, 0:TH], zbs[:])

                def apply_mask(h, P):
                    nc.vector.tensor_mul(P[:, 0:(KT - 1) * TH], P[:, 0:(KT - 1) * TH],
                                         msk[:, h * (KT - 1) * TH:(h + 1) * (KT - 1) * TH])
                    nc.vector.tensor_mul(P[:, (KT - 1) * TH:KT * TH],
                                         P[:, (KT - 1) * TH:KT * TH], m8c[:])

                for f in range(NH // 2):
                    # heads 2f (rows 0:64) and 2f+1 (rows 64:128): alternating
                    # row groups lets the PE pull LDWEIGHTS ahead of matmuls.
                    qa = Qt[0:64, f * T + base: f * T + base + TH]
                    qb = Qt[64:128, f * T + base: f * T + base + TH]
                    Pa = ppool.tile([128, KT * TH], BF16, tag="P")
                    Pb = ppool.tile([128, KT * TH], BF16, tag="P")
                    for g in range(3):
                        pqa = psq.tile([128, 512], F32, tag="qk")
                        pqb = psq.tile([128, 512], F32, tag="qk")
                        for j in range(3):
                            m = g * 3 + j
                            ka = Ktf[b][0:64, f * KPAD + m * 128: f * KPAD + (m + 1) * 128]
                            kb_ = Ktf[b][64:128, f * KPAD + m * 128: f * KPAD + (m + 1) * 128]
                            nc.tensor.matmul(pqa[:, j * TH:(j + 1) * TH], ka, qa,
                                             start=True, stop=True)
                            nc.tensor.matmul(pqb[:, j * TH:(j + 1) * TH], kb_, qb,
                                             start=True, stop=True)
                        nc.scalar.activation(Pa[:, g * 3 * TH:(g + 1) * 3 * TH],
                                             pqa[:, 0:3 * TH], AF.Exp, scale=float(SC))
                        nc.scalar.activation(Pb[:, g * 3 * TH:(g + 1) * 3 * TH],
                                             pqb[:, 0:3 * TH], AF.Exp, scale=float(SC))
                    apply_mask(2 * f, Pa)
                    apply_mask(2 * f + 1, Pb)
                    if pend is not None:
                        emit_pv(*pend[0])
                        emit_pv(*pend[1])
                    pend = ((2 * f, Pa), (2 * f + 1, Pb))
                emit_pv(*pend[0])
                emit_pv(*pend[1])

            def kvq_part0(i, kslab, vslab, bvs):
                lc = i * LNP
                layernorm(xn, 0, TH)
                proj_half(kslab, Ktl, lc + 6, 0, TH, half_major=True)
                vproj_half(vslab, bvs, 0)
                bounce(0)
                gather(0)

            def kvq_part1(i, kslab, vslab, bvs, qslab):
                lc = i * LNP
                layernorm(xn, TH, TH)
                proj_half(qslab, Qt, lc + 0, 0, TH)   # covers LN(b1) stat chain
                proj_half(kslab, Ktl, lc + 6, TH, TH, half_major=True)
                vproj_half(vslab, bvs, 1)
                bounce(1)
                gather(1)
                proj_half(qslab, Qt, lc + 0, TH, TH)

            # ---- embedding (fused) -----------------------------------------
            pslab = load_slab(pw_in, D)
            pes = wpool.tile([128, DT * VCOLS], BF16, tag="slab")
            nc.sync.dma_start(out=pes[:, 0:DT * T], in_=pe_in[:, :])
            for o in range(DT):
                psm = psa.tile([128, 512], F32, tag="acc")
                for t in range(DT):
                    nc.tensor.matmul(psm[:, 0:T],
                                     pslab[:, t * D + o * 128: t * D + (o + 1) * 128],
                                     pes[:, t * T:(t + 1) * T],
                                     start=(t == 0), stop=(t == DT - 1))
                addt = work.tile([128, T], F32, tag="wores")
                nc.sync.dma_start(out=addt[:], in_=add_in[:, o * T:(o + 1) * T])
                nc.vector.tensor_add(X[:, o * T:(o + 1) * T], psm[:, 0:T], addt[:])

            def load_kvq_weights(i):
                kslab = load_slab(wk[i], D)
                vslab = load_slab(wv[i], VCOLS)
                bvs = bvpool.tile([128, VCOLS], BF16, tag="bv")
                nc.sync.dma_start(out=bvs[:], in_=bvb[i][:, :])
                qslab = load_slab(wq[i], D)
                return kslab, vslab, bvs, qslab

            def w2_half(i, b):
                lc = i * LNP
                for o in range(DT):
                    slab = w2pool.tile([128, F], BF16, tag="w2c")
                    nc.sync.dma_start(out=slab[:], in_=w2[i][:, o * F:(o + 1) * F])
                    psm = psa.tile([128, 512], F32, tag="acc")
                    for t in range(FT):
                        nc.tensor.matmul(psm[:, 0:TH], slab[:, t * 128:(t + 1) * 128],
                                         hsb[:, t * T + b * TH:t * T + b * TH + TH],
                                         start=(t == 0), stop=(t == FT - 1))
                    tmp = work.tile([128, T], F32, tag="wores")
                    nc.vector.tensor_scalar(
                        tmp[:, 0:TH], psm[:, 0:TH],
                        lnp_sb[:, lc + 18 + o:lc + 19 + o], None, op0=ALU.add)
                    nc.vector.tensor_add(
                        X[:, o * T + b * TH:o * T + b * TH + TH],
                        X[:, o * T + b * TH:o * T + b * TH + TH].bitcast(F32),
                        tmp[:, 0:TH])

            # ---- prologue --------------------------------------------------
            kw0 = load_kvq_weights(0)
            kvq_part0(0, kw0[0], kw0[1], kw0[2])
            kvq_part1(0, kw0[0], kw0[1], kw0[2], kw0[3])

            # ---- layers ----------------------------------------------------
            for i in range(nlayers):
                lc = i * LNP
                assemble(0)
                assemble(1)
                attention(0)
                attention(1)
                # Wo + residual (fused)
                oslab = load_slab(wo[i], D)
                for o in range(DT):
                    psm = psa.tile([128, 512], F32, tag="acc")
                    for t in range(DT):
                        nc.tensor.matmul(psm[:, 0:T],
                                         oslab[:, t * D + o * 128: t * D + (o + 1) * 128],
                                         ctx[:, t * T:(t + 1) * T],
                                         start=(t == 0), stop=(t == DT - 1))
                    tmp = work.tile([128, T], F32, tag="wores")
                    nc.scalar.activation(tmp[:], psm[:, 0:T], AF.Identity,
                                         bias=lnp_sb[:, lc + 12 + o:lc + 13 + o])
                    nc.vector.tensor_add(X[:, o * T:(o + 1) * T],
                                         X[:, o * T:(o + 1) * T].bitcast(F32), tmp[:])
                # LN2 + FFN (fused)
                layernorm(xn, 0, T)
                for c in range(FT // 4):
                    slab = w1pool.tile([128, 4 * D], BF16, tag="w1c")
                    nc.sync.dma_start(out=slab[:], in_=w1[i][:, c * 4 * D:(c + 1) * 4 * D])
                    for oo in range(4):
                        o = c * 4 + oo
                        psm = psa.tile([128, 512], F32, tag="acc")
                        for t in range(DT):
                            nc.tensor.matmul(psm[:, 0:T],
                                             slab[:, oo * D + t * 128: oo * D + (t + 1) * 128],
                                             xn[:, t * T:(t + 1) * T],
                                             start=(t == 0), stop=(t == DT - 1))
                        nc.scalar.activation(hsb[:, o * T:(o + 1) * T], psm[:, 0:T],
                                             AF.Gelu,
                                             bias=lnp_sb[:, lc + 24 + o:lc + 25 + o])
                if i + 1 < nlayers:
                    kw = load_kvq_weights(i + 1)
                    w2_half(i, 0)
                    kvq_part0(i + 1, kw[0], kw[1], kw[2])
                    w2_half(i, 1)   # hides AllGather 0
                    kvq_part1(i + 1, kw[0], kw[1], kw[2], kw[3])
                else:
                    w2_half(i, 0)
                    w2_half(i, 1)

            # ---- final LN -> out -----------------------------------------
            xout = nc.alloc_sbuf_tensor("xout", [128, DT * T], F32)
            layernorm(xout, 0, T, final=True)
            nc.sync.dma_start(out=out_t[:, :], in_=xout[:])

    nc.compile()
    return nc


# ---------------------------------------------------------------- host prep

def _ft_pack(a):
    """[768, T] -> [128, 6*T] ft-tile-major."""
    Tn = a.shape[1]
    return a.reshape(DT, 128, Tn).transpose(1, 0, 2).reshape(128, DT * Tn)


def _pp_pack(v):
    return np.ascontiguousarray(v.reshape(-1, 128).T)


def build_masks(rand_attn):
    """Per core r (8 ranks): mask [128, NH*(KT-1)*TH] over the core's 136 q."""
    ra = np.asarray(rand_attn)
    cnt = np.zeros((NH, NBLK, NBLK), dtype=np.float32)
    cnt[:, 0, :] = 1.0
    cnt[:, 16, :] = 1.0
    for h in range(NH):
        for l in range(1, 16):
            base = {0, 16, l - 1, l, l + 1} if 1 < l < 15 else (
                {0, 1, 2, 16} if l == 1 else {0, 14, 15, 16})
            for j in base:
                cnt[h, l, j] += 1.0
            for r in range(R):
                cnt[h, l, int(ra[h, l - 1, r])] += 1.0
    kvalid = np.zeros((KPAD,), dtype=np.float32)
    kvalid[:SEQ] = 1.0
    masks = []
    for r in range(8):
        qg = np.arange(r * TH, (r + 1) * TH)
        lq = np.minimum(qg // BS, NBLK - 1)
        kg = np.arange(KPAD)
        jk = np.minimum(kg // BS, NBLK - 1)
        m = np.zeros((NH, KPAD, TH), dtype=BF)
        for h in range(NH):
            mh = cnt[h].T[np.ix_(jk, lq)] * kvalid[:, None]
            m[h] = mh.astype(BF)
        m = m.reshape(NH, KT, 128, TH)[:, :KT - 1]
        m = m.transpose(2, 0, 1, 3).reshape(128, NH * (KT - 1) * TH)
        masks.append(np.ascontiguousarray(m))
    return masks


def prepare_inputs(inputs, nlayers=NLAYERS):
    pv = np.asarray(inputs["pixel_values"], np.float32)
    B = pv.shape[0]
    g_img = pv.shape[2] // 16
    ntok_img = g_img * g_img
    patches = pv.reshape(B, 3, g_img, 16, g_img, 16).transpose(0, 2, 4, 1, 3, 5)
    patches = patches.reshape(B, ntok_img, 768)

    pos = np.asarray(inputs["pos_emb"], np.float32)[0]
    cls = np.asarray(inputs["cls_token"], np.float32).reshape(768)
    patch_b = np.asarray(inputs["patch_b"], np.float32)

    pzt = np.zeros((B, 768, SEQP), np.float32)
    addt = np.zeros((B, 768, SEQP), np.float32)
    for b in range(B):
        pzt[b, :, 1:1 + ntok_img] = patches[b].T
        addt[b, :, 0] = cls + pos[0]
        addt[b, :, 1:SEQ] = (patch_b[None, :] + pos[1:SEQ]).T

    masks = build_masks(inputs["rand_attn"])

    def bfc(x):
        return np.ascontiguousarray(np.asarray(x, np.float32).astype(BF))

    shared = {"pw": bfc(inputs["patch_w"])}
    normp = np.concatenate(
        [_pp_pack(np.asarray(inputs["norm_g"], np.float32)),
         _pp_pack(np.asarray(inputs["norm_b"], np.float32))], axis=1)
    shared["normp"] = np.ascontiguousarray(normp)
    lnp_all = np.zeros((128, LNP * nlayers), np.float32)
    for i in range(nlayers):
        g1 = np.asarray(inputs["ln1_g"][i], np.float32)
        b1 = np.asarray(inputs["ln1_b"][i], np.float32)
        g2 = np.asarray(inputs["ln2_g"][i], np.float32)
        b2 = np.asarray(inputs["ln2_b"][i], np.float32)
        wq_i = np.asarray(inputs["Wq"][i], np.float32)
        wk_i = np.asarray(inputs["Wk"][i], np.float32)
        wv_i = np.asarray(inputs["Wv"][i], np.float32)
        w1_i = np.asarray(inputs["ff_w1"][i], np.float32)
        shared[f"wq{i}"] = bfc(g1[:, None] * wq_i)
        shared[f"wk{i}"] = bfc(g1[:, None] * wk_i)
        shared[f"wo{i}"] = bfc(inputs["Wo"][i])
        bq_h = np.asarray(inputs["bq"][i], np.float32) + wq_i.T @ b1
        bk_h = np.asarray(inputs["bk"][i], np.float32) + wk_i.T @ b1
        b1_h = np.asarray(inputs["ff_b1"][i], np.float32) + w1_i.T @ b2
        wva = np.zeros((768, VCOLS), np.float32)
        for h in range(NH):
            wva[:, h * 65:h * 65 + 64] = g1[:, None] * wv_i[:, h * 64:(h + 1) * 64]
        shared[f"wv{i}"] = bfc(wva)
        w1g = g2[:, None] * w1_i
        w1t = w1g.reshape(DT, 128, FT, 128).transpose(1, 2, 0, 3).reshape(128, FT * D)
        shared[f"w1{i}"] = bfc(w1t)
        w2_i = np.asarray(inputs["ff_w2"][i], np.float32)
        w2t = w2_i.reshape(FT, 128, DT, 128).transpose(1, 2, 0, 3).reshape(128, DT * F)
        shared[f"w2{i}"] = bfc(w2t)
        lnp_all[:, i * LNP + 0:i * LNP + 6] = _pp_pack(bq_h)
        lnp_all[:, i * LNP + 6:i * LNP + 12] = _pp_pack(bk_h)
        lnp_all[:, i * LNP + 12:i * LNP + 18] = _pp_pack(
            np.asarray(inputs["bo"][i], np.float32))
        lnp_all[:, i * LNP + 18:i * LNP + 24] = _pp_pack(
            np.asarray(inputs["ff_b2"][i], np.float32))
        lnp_all[:, i * LNP + 24:i * LNP + 48] = _pp_pack(b1_h)
        bva = np.zeros((VCOLS,), np.float32)
        bv_i = np.asarray(inputs["bv"][i], np.float32) + wv_i.T @ b1
        for h in range(NH):
            bva[h * 65:h * 65 + 64] = bv_i[h * 64:(h + 1) * 64]
            bva[h * 65 + 64] = 1.0
        shared[f"bvb{i}"] = np.ascontiguousarray(
            np.broadcast_to(bva.astype(BF), (128, VCOLS)))
    shared["lnp"] = np.ascontiguousarray(lnp_all)

    in_maps = []
    for c in range(8):
        im = dict(shared)
        sl = slice(c * TH, (c + 1) * TH)
        pe_c = np.concatenate([pzt[0][:, sl], pzt[1][:, sl]], axis=1)   # [768, 272]
        ad_c = np.concatenate([addt[0][:, sl], addt[1][:, sl]], axis=1)
        im["pe_in"] = np.ascontiguousarray(_ft_pack(pe_c).astype(BF))
        im["add_in"] = np.ascontiguousarray(_ft_pack(ad_c))
        im["mask_in"] = masks[c]
        in_maps.append(im)
    return in_maps


LAST_RESULT = None


def kernel(**inputs):
    global LAST_RESULT
    key = ("prog", NLAYERS)
    if key not in _CACHE:
        _CACHE[key] = build_program(NLAYERS)
    nc = _CACHE[key]
    in_maps = prepare_inputs(inputs, NLAYERS)
    kw = {}
    if os.environ.get("BB_TRACE", "0") == "1":
        kw = dict(trace=True, tmpdir=os.environ.get("BB_TRACE_DIR") or None)
    res = run_bass_kernel_spmd(nc, in_maps, core_ids=list(range(8)), **kw)
    LAST_RESULT = res
    full = [np.zeros((768, SEQP), np.float32) for _ in range(2)]
    for c in range(8):
        o = res.results[c]["out"]                      # [128, 6*T]
        o = o.reshape(128, DT, T).transpose(1, 0, 2).reshape(768, T)
        full[0][:, c * TH:(c + 1) * TH] = o[:, 0:TH]
        full[1][:, c * TH:(c + 1) * TH] = o[:, TH:2 * TH]
    return np.stack([f[:, :SEQ].T for f in full], axis=0).astype(np.float32)


if __name__ == "__main__":
    import reference
    ins = {k: np.asarray(v) for k, v in reference.setup_inputs().items()}
    got = kernel(**ins)
    print("kernel output", got.shape)


# revision 20
# speedup vs baseline: 1.2010x; 1.2010x over previous
"""BigBird ViT forward on 8 Trainium2 NeuronCores — half-staggered attention.

Sharding: every core holds 136 tokens of BOTH batch elements (8-way token
sharding per element). Dense compute (LN / projections / FFN) runs fused over
all 272 resident token columns; K/V projections + the per-element fused K/V
AllGather run per batch element and are staggered so each AllGather hides
behind the other element's projections / Q / attention.

On-chip layout is transposed ([feature, token]); each 272-wide token axis is
[136 tokens of b0 | 136 tokens of b1]. LayerNorm reductions run as a single
f32r ones-matmul; rstd = exp(-0.5*ln(var+eps)). LN gain/bias are folded into
the following projections host-side. BigBird band/random/global structure is
a multiplicative {0,1,2} bf16 mask. Attention is software-pipelined: head
h+1's QK matmuls are emitted before head h's PV so the PE never waits on the
exp/mask chain.
"""
import os
import sys

sys.path.insert(0, "/opt/trn_rl_repo")

import numpy as np
import ml_dtypes

import concourse.bass as bass
import concourse.bacc as bacc
import concourse.mybir as mybir
import concourse.tile as tile
from concourse.bass_utils import run_bass_kernel_spmd

F32 = mybir.dt.float32
F32R = mybir.dt.float32r
BF16 = mybir.dt.bfloat16
F8 = mybir.dt.float8e4
AF = mybir.ActivationFunctionType
ALU = mybir.AluOpType
BF = ml_dtypes.bfloat16

# model dims
BS = 64; NH = 12; HD = 64; D = 768; F = 3072; L = 12; R = 3
SEQ = 1025
SEQP = 1088           # padded to 17 blocks of 64
NBLK = 17
TH = SEQP // 8        # tokens per core per batch element = 136
T = 2 * TH            # token columns per core = 272 (b0 | b1)
DT = D // 128         # 6 feature tiles
FT = F // 128         # 24 ffn tiles
KT = 9                # k tiles over 1152 (1088 padded up; tile 8 is half real)
KPAD = 1152
VCOLS = NH * (HD + 1)  # 780: per-head [64 V cols + 1 ones col]
SC = 1.0 / np.sqrt(HD)
LNP = 48              # bias columns per layer
G8 = [[0, 1, 2, 3, 4, 5, 6, 7]]
KB = DT * TH          # 816: K bounce cols per half

NLAYERS = int(os.environ.get("BB_NLAYERS", str(L)))

_CACHE = {}


# ---------------------------------------------------------------- builder

def build_program(nlayers=NLAYERS):
    nc = bacc.Bacc("TRN2", target_bir_lowering=False, debug=False, num_devices=8)

    # ---- DRAM I/O -------------------------------------------------------
    pe_in = nc.dram_tensor("pe_in", [128, DT * T], BF16, kind="ExternalInput")
    add_in = nc.dram_tensor("add_in", [128, DT * T], F32, kind="ExternalInput")
    mask_in = nc.dram_tensor("mask_in", [128, NH * (KT - 1) * TH], BF16, kind="ExternalInput")
    pw_in = nc.dram_tensor("pw", [D, D], BF16, kind="ExternalInput")
    normp_in = nc.dram_tensor("normp", [128, 2 * DT], F32, kind="ExternalInput")
    lnp_in = nc.dram_tensor("lnp", [128, LNP * nlayers], F32, kind="ExternalInput")
    wq = [nc.dram_tensor(f"wq{i}", [D, D], BF16, kind="ExternalInput") for i in range(nlayers)]
    wk = [nc.dram_tensor(f"wk{i}", [D, D], BF16, kind="ExternalInput") for i in range(nlayers)]
    wv = [nc.dram_tensor(f"wv{i}", [D, VCOLS], BF16, kind="ExternalInput") for i in range(nlayers)]
    wo = [nc.dram_tensor(f"wo{i}", [D, D], BF16, kind="ExternalInput") for i in range(nlayers)]
    w1 = [nc.dram_tensor(f"w1{i}", [128, FT * D], BF16, kind="ExternalInput") for i in range(nlayers)]
    w2 = [nc.dram_tensor(f"w2{i}", [128, DT * F], BF16, kind="ExternalInput") for i in range(nlayers)]
    bvb = [nc.dram_tensor(f"bvb{i}", [128, VCOLS], BF16, kind="ExternalInput") for i in range(nlayers)]
    out_t = nc.dram_tensor("out", [128, DT * T], F32, kind="ExternalOutput")

    KV_K = D * TH
    KV_V = TH * VCOLS
    KV = KV_K + KV_V
    kvi = [nc.dram_tensor(f"kv_in{b}", [KV], F8) for b in range(2)]
    kvo = [nc.dram_tensor(f"kv_out{b}", [8 * KV], F8, addr_space="Shared")
           for b in range(2)]

    with tile.TileContext(nc) as tc:
        # ---- persistent SBUF tensors -----------------------------------
        X = nc.alloc_sbuf_tensor("X", [128, DT * T], F32R)
        xn = nc.alloc_sbuf_tensor("xn", [128, DT * T], BF16)
        Qt = nc.alloc_sbuf_tensor("Qt", [128, DT * T], BF16)
        Ktl = nc.alloc_sbuf_tensor("Ktl", [128, 2 * KB], F8)   # [half][t][j]
        Vnl = nc.alloc_sbuf_tensor("Vnl", [128, 4 * VCOLS], F8)  # [half][tile 128|8]
        Ktf = [nc.alloc_sbuf_tensor(f"Ktf{b}", [128, DT * KPAD], F8) for b in range(2)]
        Vnf = [nc.alloc_sbuf_tensor(f"Vnf{b}", [128, KT * VCOLS], F8) for b in range(2)]
        ctx = nc.alloc_sbuf_tensor("ctx", [128, DT * T], BF16)
        hsb = nc.alloc_sbuf_tensor("hsb", [128, FT * T], BF16)
        msk = nc.alloc_sbuf_tensor("msk", [128, NH * (KT - 1) * TH], BF16)
        m8c = nc.alloc_sbuf_tensor("m8c", [128, TH], BF16)
        onesr = nc.alloc_sbuf_tensor("onesr", [128, 1], F32R)
        onesf = nc.alloc_sbuf_tensor("onesf", [128, 1], F32)
        lnp_sb = nc.alloc_sbuf_tensor("lnp_sb", [128, LNP * nlayers], F32)
        normp_sb = nc.alloc_sbuf_tensor("normp_sb", [128, 2 * DT], F32)
        epsb = nc.alloc_sbuf_tensor("epsb", [1, 1], F32)

        with (
            tc.tile_pool(name="wpool", bufs=3) as wpool,
            tc.tile_pool(name="w1pool", bufs=2) as w1pool,
            tc.tile_pool(name="w2pool", bufs=2) as w2pool,
            tc.tile_pool(name="bvpool", bufs=2) as bvpool,
            tc.tile_pool(name="work", bufs=2) as work,
            tc.tile_pool(name="stat", bufs=2) as stat,
            tc.tile_pool(name="ppool", bufs=4) as ppool,
            tc.tile_pool(name="psq", bufs=6, space="PSUM") as psq,
            tc.tile_pool(name="psa", bufs=2, space="PSUM") as psa,
        ):
            nc.vector.memset(onesf[:], 1.0)
            nc.vector.tensor_copy(onesr[:], onesf[:])
            nc.vector.memset(epsb[:], 1e-5)
            for b in range(2):
                for t in range(DT):
                    nc.vector.memset(Ktf[b][:, t * KPAD + SEQP: (t + 1) * KPAD], 0.0)
                nc.vector.memset(Vnf[b][64:128, (KT - 1) * VCOLS: KT * VCOLS], 0.0)
            nc.sync.dma_start(out=msk[:], in_=mask_in[:, :])
            nc.sync.dma_start(out=lnp_sb[:], in_=lnp_in[:, :])
            nc.sync.dma_start(out=normp_sb[:], in_=normp_in[:, :])
            nc.vector.memset(m8c[:], 0.0)
            nc.vector.memset(m8c[0:1, :], 1.0)

            def layernorm(out_sb, base, w, final=False):
                """fused LN over features for token cols [base, base+w).

                xn = X*rstd - mu*rstd; the elementwise work is split between
                the Vector and GpSimd engines; mu*rstd | rstd are broadcast in
                one shot to shorten the stat-chain latency.
                """
                sum_ps = psa.tile([1, 512], F32, tag="acc")
                sq_ps = psa.tile([1, 512], F32, tag="acc")
                for t in range(DT):
                    nc.tensor.matmul(sum_ps[:, 0:w], onesr[:],
                                     X[:, t * T + base: t * T + base + w],
                                     start=(t == 0), stop=(t == DT - 1))
                for t in range(DT):
                    sq = work.tile([128, T], F32R, tag="lnsq")
                    s32 = X[:, t * T + base: t * T + base + w].bitcast(F32)
                    nc.vector.tensor_mul(sq[:, 0:w], s32, s32)
                    nc.tensor.matmul(sq_ps[:, 0:w], onesr[:], sq[:, 0:w],
                                     start=(t == 0), stop=(t == DT - 1))
                st = stat.tile([1, 2 * T], F32, tag="mr")   # [mu*rstd | rstd]
                va = stat.tile([1, T], F32, tag="var")
                nc.scalar.activation(st[:, 0:w], sum_ps[:, 0:w], AF.Identity, scale=1.0 / D)
                nc.scalar.activation(va[:, 0:w], sq_ps[:, 0:w], AF.Identity, scale=1.0 / D)
                mu2 = stat.tile([1, T], F32, tag="mu2")
                nc.vector.tensor_mul(mu2[:, 0:w], st[:, 0:w], st[:, 0:w])
                nc.vector.tensor_sub(va[:, 0:w], va[:, 0:w], mu2[:, 0:w])
                nc.scalar.activation(va[:, 0:w], va[:, 0:w], AF.Ln, bias=epsb[0:1, 0:1])
                nc.scalar.activation(st[:, w:2 * w], va[:, 0:w], AF.Exp, scale=-0.5)
                nc.vector.tensor_mul(st[:, 0:w], st[:, 0:w], st[:, w:2 * w])
                mr2 = work.tile([128, 2 * T], F32, tag="mrb")
                nc.gpsimd.partition_broadcast(mr2[:, 0:2 * w], st[:, 0:2 * w])
                for t in range(DT):
                    tmp = work.tile([128, T], F32, tag="lntmp")
                    nc.vector.tensor_mul(tmp[:, 0:w],
                                         X[:, t * T + base: t * T + base + w].bitcast(F32),
                                         mr2[:, w:2 * w])
                    if final:
                        nc.vector.tensor_sub(tmp[:, 0:w], tmp[:, 0:w], mr2[:, 0:w])
                        nc.vector.tensor_scalar(
                            out_sb[:, t * T + base: t * T + base + w], tmp[:, 0:w],
                            normp_sb[:, t:t + 1], normp_sb[:, DT + t:DT + t + 1],
                            op0=ALU.mult, op1=ALU.add)
                    else:
                        nc.vector.tensor_sub(out_sb[:, t * T + base: t * T + base + w],
                                             tmp[:, 0:w], mr2[:, 0:w])

            def load_slab(wdram, ncols):
                slab = wpool.tile([128, DT * VCOLS], BF16, tag="slab")
                nc.sync.dma_start(
                    out=slab[:, 0:DT * ncols].rearrange("p (t c) -> p t c", c=ncols),
                    in_=wdram[:, :].rearrange("(t p) c -> p t c", p=128))
                return slab

            def proj_half(slab, out_sb, bias_col, base, w, half_major=False):
                """out[:, o-tile cols base..base+w] = slab.T @ xn cols."""
                for o in range(DT):
                    psm = psa.tile([128, 512], F32, tag="acc")
                    for t in range(DT):
                        nc.tensor.matmul(psm[:, 0:w],
                                         slab[:, t * D + o * 128: t * D + (o + 1) * 128],
                                         xn[:, t * T + base: t * T + base + w],
                                         start=(t == 0), stop=(t == DT - 1))
                    if half_major:
                        dst = out_sb[:, base * DT + o * w: base * DT + (o + 1) * w]
                    else:
                        dst = out_sb[:, o * T + base: o * T + base + w]
                    nc.vector.tensor_scalar(
                        dst, psm[:, 0:w],
                        lnp_sb[:, bias_col + o:bias_col + o + 1], None,
                        op0=ALU.add)

            def vproj_half(vslab, bvs, b):
                """V proj for half b into Vnl tiles (2b: rows0-127, 2b+1: rows 0-7)."""
                base = b * TH
                for s, rows in ((0, 128), (1, TH - 128)):
                    for half in range(2):
                        n0, n1 = half * 390, (half + 1) * 390
                        psm = psa.tile([128, 512], F32, tag="acc")
                        for t in range(DT):
                            nc.tensor.matmul(
                                psm[:rows, 0:390],
                                xn[:, t * T + base + s * 128: t * T + base + s * 128 + rows],
                                vslab[:, t * VCOLS + n0: t * VCOLS + n1],
                                start=(t == 0), stop=(t == DT - 1))
                        nc.vector.tensor_add(
                            Vnl[:rows, (2 * b + s) * VCOLS + n0: (2 * b + s) * VCOLS + n1],
                            psm[:rows, 0:390], bvs[:rows, n0:n1])

            def bounce(b):
                """ship half b's K^T / V shards to the collective input."""
                nc.sync.dma_start(
                    out=kvi[b][0:KV_K].rearrange("(p x) -> p x", p=128),
                    in_=Ktl[:, b * KB:(b + 1) * KB])
                nc.sync.dma_start(
                    out=kvi[b][KV_K:KV_K + 128 * VCOLS].rearrange("(a v) -> a v", v=VCOLS),
                    in_=Vnl[0:128, 2 * b * VCOLS:(2 * b + 1) * VCOLS])
                nc.sync.dma_start(
                    out=kvi[b][KV_K + 128 * VCOLS:KV].rearrange("(a v) -> a v", v=VCOLS),
                    in_=Vnl[0:TH - 128, (2 * b + 1) * VCOLS:(2 * b + 2) * VCOLS])

            def gather(b):
                nc.gpsimd.collective_compute(
                    "AllGather", ALU.bypass, replica_groups=G8,
                    ins=[kvi[b][:].opt()], outs=[kvo[b][:].opt()])

            def assemble(b):
                for c in range(8):
                    kc = kvo[b][c * KV: c * KV + KV_K]
                    nc.sync.dma_start(
                        out=Ktf[b][:].rearrange("p (t k) -> p t k", k=KPAD)
                            [:, :, c * TH:(c + 1) * TH],
                        in_=kc.rearrange("(p t j) -> p t j", t=DT, j=TH))
                for m in range(KT):
                    r0 = m * 128
                    rows = 128 if m < KT - 1 else SEQP - r0
                    r = r0
                    while r < r0 + rows:
                        c = r // TH
                        take = min((c + 1) * TH, r0 + rows) - r
                        vc = kvo[b][c * KV + KV_K + (r - c * TH) * VCOLS:
                                    c * KV + KV_K + (r - c * TH + take) * VCOLS]
                        nc.sync.dma_start(
                            out=Vnf[b][r - r0: r - r0 + take, m * VCOLS:(m + 1) * VCOLS],
                            in_=vc.rearrange("(a v) -> a v", v=VCOLS))
                        r += take

            def attention(b):
                base = b * TH
                pend = None

                def emit_pv(h, P):
                    ft, row = h // 2, (h % 2) * 64
                    cps = psa.tile([65, 512], F32, tag="acc")
                    for m in range(KT):
                        vh = Vnf[b][:, m * VCOLS + h * 65: m * VCOLS + (h + 1) * 65]
                        nc.tensor.matmul(cps[:, 0:TH], vh, P[:, m * TH:(m + 1) * TH],
                                         start=(m == 0), stop=(m == KT - 1))
                    zrec = stat.tile([1, TH], F32, tag="zrec")
                    nc.vector.reciprocal(zrec[:], cps[64:65, 0:TH])
                    zbs = work.tile([64, TH], F32, tag="zbs")
                    nc.gpsimd.partition_broadcast(zbs[:], zrec[:])
                    nc.vector.tensor_mul(ctx[row:row + 64, ft * T + base: ft * T + base + TH],
                                         cps[0:64, 0:TH], zbs[:])

                def apply_mask(h, P):
                    nc.vector.tensor_mul(P[:, 0:(KT - 1) * TH], P[:, 0:(KT - 1) * TH],
                                         msk[:, h * (KT - 1) * TH:(h + 1) * (KT - 1) * TH])
                    nc.vector.tensor_mul(P[:, (KT - 1) * TH:KT * TH],
                                         P[:, (KT - 1) * TH:KT * TH], m8c[:])

                for f in range(NH // 2):
                    # heads 2f (rows 0:64) and 2f+1 (rows 64:128): alternating
                    # row groups lets the PE pull LDWEIGHTS ahead of matmuls.
                    qa = Qt[0:64, f * T + base: f * T + base + TH]
                    qb = Qt[64:128, f * T + base: f * T + base + TH]
                    Pa = ppool.tile([128, KT * TH], BF16, tag="P")
                    Pb = ppool.tile([128, KT * TH], BF16, tag="P")
                    for g in range(3):
                        pqa = psq.tile([128, 512], F32, tag="qk")
                        pqb = psq.tile([128, 512], F32, tag="qk")
                        for j in range(3):
                            m = g * 3 + j
                            ka = Ktf[b][0:64, f * KPAD + m * 128: f * KPAD + (m + 1) * 128]
                            kb_ = Ktf[b][64:128, f * KPAD + m * 128: f * KPAD + (m + 1) * 128]
                            nc.tensor.matmul(pqa[:, j * TH:(j + 1) * TH], ka, qa,
                                             start=True, stop=True)
                            nc.tensor.matmul(pqb[:, j * TH:(j + 1) * TH], kb_, qb,
                                             start=True, stop=True)
                        nc.scalar.activation(Pa[:, g * 3 * TH:(g + 1) * 3 * TH],
                                             pqa[:, 0:3 * TH], AF.Exp, scale=float(SC))
                        nc.scalar.activation(Pb[:, g * 3 * TH:(g + 1) * 3 * TH],
                                             pqb[:, 0:3 * TH], AF.Exp, scale=float(SC))
                    apply_mask(2 * f, Pa)
                    apply_mask(2 * f + 1, Pb)
                    if pend is not None:
                        emit_pv(*pend[0])
                        emit_pv(*pend[1])
                    pend = ((2 * f, Pa), (2 * f + 1, Pb))
                emit_pv(*pend[0])
                emit_pv(*pend[1])

            def kvq_part0(i, kslab, vslab, bvs):
                lc = i * LNP
                layernorm(xn, 0, TH)
                proj_half(kslab, Ktl, lc + 6, 0, TH, half_major=True)
                vproj_half(vslab, bvs, 0)
                bounce(0)
                gather(0)

            def kvq_part1(i, kslab, vslab, bvs, qslab):
                lc = i * LNP
                layernorm(xn, TH, TH)
                proj_half(qslab, Qt, lc + 0, 0, TH)   # covers LN(b1) stat chain
                proj_half(kslab, Ktl, lc + 6, TH, TH, half_major=True)
                vproj_half(vslab, bvs, 1)
                bounce(1)
                gather(1)
                proj_half(qslab, Qt, lc + 0, TH, TH)

            # ---- embedding (fused) -----------------------------------------
            pslab = load_slab(pw_in, D)
            pes = wpool.tile([128, DT * VCOLS], BF16, tag="slab")
            nc.sync.dma_start(out=pes[:, 0:DT * T], in_=pe_in[:, :])
            for o in range(DT):
                psm = psa.tile([128, 512], F32, tag="acc")
                for t in range(DT):
                    nc.tensor.matmul(psm[:, 0:T],
                                     pslab[:, t * D + o * 128: t * D + (o + 1) * 128],
                                     pes[:, t * T:(t + 1) * T],
                                     start=(t == 0), stop=(t == DT - 1))
                addt = work.tile([128, T], F32, tag="wores")
                nc.sync.dma_start(out=addt[:], in_=add_in[:, o * T:(o + 1) * T])
                nc.vector.tensor_add(X[:, o * T:(o + 1) * T], psm[:, 0:T], addt[:])

            def load_kvq_weights(i):
                kslab = load_slab(wk[i], D)
                vslab = load_slab(wv[i], VCOLS)
                bvs = bvpool.tile([128, VCOLS], BF16, tag="bv")
                nc.sync.dma_start(out=bvs[:], in_=bvb[i][:, :])
                qslab = load_slab(wq[i], D)
                return kslab, vslab, bvs, qslab

            def w2_half(i, b):
                lc = i * LNP
                for o in range(DT):
                    slab = w2pool.tile([128, F], BF16, tag="w2c")
                    nc.sync.dma_start(out=slab[:], in_=w2[i][:, o * F:(o + 1) * F])
                    psm = psa.tile([128, 512], F32, tag="acc")
                    for t in range(FT):
                        nc.tensor.matmul(psm[:, 0:TH], slab[:, t * 128:(t + 1) * 128],
                                         hsb[:, t * T + b * TH:t * T + b * TH + TH],
                                         start=(t == 0), stop=(t == FT - 1))
                    tmp = work.tile([128, T], F32, tag="wores")
                    nc.vector.tensor_scalar(
                        tmp[:, 0:TH], psm[:, 0:TH],
                        lnp_sb[:, lc + 18 + o:lc + 19 + o], None, op0=ALU.add)
                    nc.vector.tensor_add(
                        X[:, o * T + b * TH:o * T + b * TH + TH],
                        X[:, o * T + b * TH:o * T + b * TH + TH].bitcast(F32),
                        tmp[:, 0:TH])

            # ---- prologue --------------------------------------------------
            kw0 = load_kvq_weights(0)
            kvq_part0(0, kw0[0], kw0[1], kw0[2])
            kvq_part1(0, kw0[0], kw0[1], kw0[2], kw0[3])

            # ---- layers ----------------------------------------------------
            for i in range(nlayers):
                lc = i * LNP
                assemble(0)
                assemble(1)
                attention(0)
                attention(1)
                # Wo + residual (fused)
                oslab = load_slab(wo[i], D)
                for o in range(DT):
                    psm = psa.tile([128, 512], F32, tag="acc")
                    for t in range(DT):
                        nc.tensor.matmul(psm[:, 0:T],
                                         oslab[:, t * D + o * 128: t * D + (o + 1) * 128],
                                         ctx[:, t * T:(t + 1) * T],
                                         start=(t == 0), stop=(t == DT - 1))
                    tmp = work.tile([128, T], F32, tag="wores")
                    nc.scalar.activation(tmp[:], psm[:, 0:T], AF.Identity,
                                         bias=lnp_sb[:, lc + 12 + o:lc + 13 + o])
                    nc.vector.tensor_add(X[:, o * T:(o + 1) * T],
                                         X[:, o * T:(o + 1) * T].bitcast(F32), tmp[:])
                # LN2 + FFN (fused)
                layernorm(xn, 0, T)
                for c in range(FT // 4):
                    slab = w1pool.tile([128, 4 * D], BF16, tag="w1c")
                    nc.sync.dma_start(out=slab[:], in_=w1[i][:, c * 4 * D:(c + 1) * 4 * D])
                    for oo in range(4):
                        o = c * 4 + oo
                        psm = psa.tile([128, 512], F32, tag="acc")
                        for t in range(DT):
                            nc.tensor.matmul(psm[:, 0:T],
                                             slab[:, oo * D + t * 128: oo * D + (t + 1) * 128],
                                             xn[:, t * T:(t + 1) * T],
                                             start=(t == 0), stop=(t == DT - 1))
                        nc.scalar.activation(hsb[:, o * T:(o + 1) * T], psm[:, 0:T],
                                             AF.Gelu,
                                             bias=lnp_sb[:, lc + 24 + o:lc + 25 + o])
                if i + 1 < nlayers:
                    kw = load_kvq_weights(i + 1)
                    w2_half(i, 0)
                    kvq_part0(i + 1, kw[0], kw[1], kw[2])
                    w2_half(i, 1)   # hides AllGather 0
                    kvq_part1(i + 1, kw[0], kw[1], kw[2], kw[3])
                else:
                    w2_half(i, 0)
                    w2_half(i, 1)

            # ---- final LN -> out -----------------------------------------
            xout = nc.alloc_sbuf_tensor("xout", [128, DT * T], F32)
            layernorm(xout, 0, T, final=True)
            nc.sync.dma_start(out=out_t[:, :], in_=xout[:])

    nc.compile()
    return nc


# ---------------------------------------------------------------- host prep

def _ft_pack(a):
    """[768, T] -> [128, 6*T] ft-tile-major."""
    Tn = a.shape[1]
    return a.reshape(DT, 128, Tn).transpose(1, 0, 2).reshape(128, DT * Tn)


def _pp_pack(v):
    return np.ascontiguousarray(v.reshape(-1, 128).T)


def build_masks(rand_attn):
    """Per core r (8 ranks): mask [128, NH*(KT-1)*TH] over the core's 136 q."""
    ra = np.asarray(rand_attn)
    cnt = np.zeros((NH, NBLK, NBLK), dtype=np.float32)
    cnt[:, 0, :] = 1.0
    cnt[:, 16, :] = 1.0
    for h in range(NH):
        for l in range(1, 16):
            base = {0, 16, l - 1, l, l + 1} if 1 < l < 15 else (
                {0, 1, 2, 16} if l == 1 else {0, 14, 15, 16})
            for j in base:
                cnt[h, l, j] += 1.0
            for r in range(R):
                cnt[h, l, int(ra[h, l - 1, r])] += 1.0
    kvalid = np.zeros((KPAD,), dtype=np.float32)
    kvalid[:SEQ] = 1.0
    masks = []
    for r in range(8):
        qg = np.arange(r * TH, (r + 1) * TH)
        lq = np.minimum(qg // BS, NBLK - 1)
        kg = np.arange(KPAD)
        jk = np.minimum(kg // BS, NBLK - 1)
        m = np.zeros((NH, KPAD, TH), dtype=BF)
        for h in range(NH):
            mh = cnt[h].T[np.ix_(jk, lq)] * kvalid[:, None]
            m[h] = mh.astype(BF)
        m = m.reshape(NH, KT, 128, TH)[:, :KT - 1]
        m = m.transpose(2, 0, 1, 3).reshape(128, NH * (KT - 1) * TH)
        masks.append(np.ascontiguousarray(m))
    return masks


def prepare_inputs(inputs, nlayers=NLAYERS):
    pv = np.asarray(inputs["pixel_values"], np.float32)
    B = pv.shape[0]
    g_img = pv.shape[2] // 16
    ntok_img = g_img * g_img
    patches = pv.reshape(B, 3, g_img, 16, g_img, 16).transpose(0, 2, 4, 1, 3, 5)
    patches = patches.reshape(B, ntok_img, 768)

    pos = np.asarray(inputs["pos_emb"], np.float32)[0]
    cls = np.asarray(inputs["cls_token"], np.float32).reshape(768)
    patch_b = np.asarray(inputs["patch_b"], np.float32)

    pzt = np.zeros((B, 768, SEQP), np.float32)
    addt = np.zeros((B, 768, SEQP), np.float32)
    for b in range(B):
        pzt[b, :, 1:1 + ntok_img] = patches[b].T
        addt[b, :, 0] = cls + pos[0]
        addt[b, :, 1:SEQ] = (patch_b[None, :] + pos[1:SEQ]).T

    masks = build_masks(inputs["rand_attn"])

    def bfc(x):
        return np.ascontiguousarray(np.asarray(x, np.float32).astype(BF))

    shared = {"pw": bfc(inputs["patch_w"])}
    normp = np.concatenate(
        [_pp_pack(np.asarray(inputs["norm_g"], np.float32)),
         _pp_pack(np.asarray(inputs["norm_b"], np.float32))], axis=1)
    shared["normp"] = np.ascontiguousarray(normp)
    lnp_all = np.zeros((128, LNP * nlayers), np.float32)
    for i in range(nlayers):
        g1 = np.asarray(inputs["ln1_g"][i], np.float32)
        b1 = np.asarray(inputs["ln1_b"][i], np.float32)
        g2 = np.asarray(inputs["ln2_g"][i], np.float32)
        b2 = np.asarray(inputs["ln2_b"][i], np.float32)
        wq_i = np.asarray(inputs["Wq"][i], np.float32)
        wk_i = np.asarray(inputs["Wk"][i], np.float32)
        wv_i = np.asarray(inputs["Wv"][i], np.float32)
        w1_i = np.asarray(inputs["ff_w1"][i], np.float32)
        shared[f"wq{i}"] = bfc(g1[:, None] * wq_i)
        shared[f"wk{i}"] = bfc(g1[:, None] * wk_i)
        shared[f"wo{i}"] = bfc(inputs["Wo"][i])
        bq_h = np.asarray(inputs["bq"][i], np.float32) + wq_i.T @ b1
        bk_h = np.asarray(inputs["bk"][i], np.float32) + wk_i.T @ b1
        b1_h = np.asarray(inputs["ff_b1"][i], np.float32) + w1_i.T @ b2
        wva = np.zeros((768, VCOLS), np.float32)
        for h in range(NH):
            wva[:, h * 65:h * 65 + 64] = g1[:, None] * wv_i[:, h * 64:(h + 1) * 64]
        shared[f"wv{i}"] = bfc(wva)
        w1g = g2[:, None] * w1_i
        w1t = w1g.reshape(DT, 128, FT, 128).transpose(1, 2, 0, 3).reshape(128, FT * D)
        shared[f"w1{i}"] = bfc(w1t)
        w2_i = np.asarray(inputs["ff_w2"][i], np.float32)
        w2t = w2_i.reshape(FT, 128, DT, 128).transpose(1, 2, 0, 3).reshape(128, DT * F)
        shared[f"w2{i}"] = bfc(w2t)
        lnp_all[:, i * LNP + 0:i * LNP + 6] = _pp_pack(bq_h)
        lnp_all[:, i * LNP + 6:i * LNP + 12] = _pp_pack(bk_h)
        lnp_all[:, i * LNP + 12:i * LNP + 18] = _pp_pack(
            np.asarray(inputs["bo"][i], np.float32))
        lnp_all[:, i * LNP + 18:i * LNP + 24] = _pp_pack(
            np.asarray(inputs["ff_b2"][i], np.float32))
        lnp_all[:, i * LNP + 24:i * LNP + 48] = _pp_pack(b1_h)
        bva = np.zeros((VCOLS,), np.float32)
        bv_i = np.asarray(inputs["bv"][i], np.float32) + wv_i.T @ b1
        for h in range(NH):
            bva[h * 65:h * 65 + 64] = bv_i[h * 64:(h + 1) * 64]
            bva[h * 65 + 64] = 1.0
        shared[f"bvb{i}"] = np.ascontiguousarray(
            np.broadcast_to(bva.astype(BF), (128, VCOLS)))
    shared["lnp"] = np.ascontiguousarray(lnp_all)

    in_maps = []
    for c in range(8):
        im = dict(shared)
        sl = slice(c * TH, (c + 1) * TH)
        pe_c = np.concatenate([pzt[0][:, sl], pzt[1][:, sl]], axis=1)   # [768, 272]
        ad_c = np.concatenate([addt[0][:, sl], addt[1][:, sl]], axis=1)
        im["pe_in"] = np.ascontiguousarray(_ft_pack(pe_c).astype(BF))
        im["add_in"] = np.ascontiguousarray(_ft_pack(ad_c))
        im["mask_in"] = masks[c]
        in_maps.append(im)
    return in_maps


LAST_RESULT = None


def kernel(**inputs):
    global LAST_RESULT
    key = ("prog", NLAYERS)
    if key not in _CACHE:
        _CACHE[key] = build_program(NLAYERS)
    nc = _CACHE[key]
    in_maps = prepare_inputs(inputs, NLAYERS)
    kw = {}
    if os.environ.get("BB_TRACE", "0") == "1":
        kw = dict(trace=True, tmpdir=os.environ.get("BB_TRACE_DIR") or None)
    res = run_bass_kernel_spmd(nc, in_maps, core_ids=list(range(8)), **kw)
    LAST_RESULT = res
    full = [np.zeros((768, SEQP), np.float32) for _ in range(2)]
    for c in range(8):
        o = res.results[c]["out"]                      # [128, 6*T]
        o = o.reshape(128, DT, T).transpose(1, 0, 2).reshape(768, T)
        full[0][:, c * TH:(c + 1) * TH] = o[:, 0:TH]
        full[1][:, c * TH:(c + 1) * TH] = o[:, TH:2 * TH]
    return np.stack([f[:, :SEQ].T for f in full], axis=0).astype(np.float32)


if __name__ == "__main__":
    import reference
    ins = {k: np.asarray(v) for k, v in reference.setup_inputs().items()}
    got = kernel(**ins)
    print("kernel output", got.shape)
